# revision 6
# baseline (speedup 1.0000x reference)
"""Bahdanau cross-attention on 8 Trainium2 NeuronCores.

Sharding: 8 cores = B(4) x NQ-halves(2). Each core owns one batch b and 64
queries, with the FULL NKV=1024 keys -> softmax is fully local, no
collectives. Host only slices inputs / concatenates outputs.

Per-core algorithm:
  - transpose kv/weights on device via PE (identity matmul), f32 data,
    PSUM->SBUF evacuation casts to bf16 for all TensorE operands
  - kpT[d=256, k=1024] = W_k @ kv^T  (PSUM f32, resident; ACT reads PSUM)
  - qpT[d=256, q=64]   = W_q @ q^T   (SBUF f32, per-partition ACT bias)
  - for each q: t[d,k] = tanh(kpT + qpT[:,q]) one ACT instr per d-block
    (bias does the broadcast add for free), t written as bf16; PE reduces
    sum_d v_d*t with t-as-stationary (bf16 LDWEIGHTS at 1 cyc/row),
    v-as-moving -> energy^T column in PSUM
  - q-loop split in two 32-query halves; each half's softmax (DVE max, ACT
    exp with bias=-max + accum_out denominator), alpha, context and output
    projection overlap the other half's tanh stream
  - out = context @ W_o^T + b_o via a ones-row matmul trick for the bias
"""

import os
import sys
from contextlib import ExitStack

import numpy as np

os.environ.setdefault("MYCRO_LOCAL_CACHE", "1")
for _p in ("/opt/trn_rl_repo", "/root/.axon_site/_ro/trn_rl_repo"):
    if os.path.isdir(_p) and _p not in sys.path:
        sys.path.append(_p)

B, NQ, NKV = 4, 128, 1024
Q_DIM, KV_DIM, ATTN_DIM = 512, 768, 256
QH = NQ // 2  # 64 queries per core
HH = QH // 2  # 32 queries per post-processing half
N_CORES = 8

_CACHE = {}
last_results = None  # BassKernelResults from the most recent run (for test.py)


def _build():
    from concourse import bacc, masks, mybir, tile

    dt = mybir.dt
    f32 = dt.float32
    bf16 = dt.bfloat16
    AF = mybir.ActivationFunctionType

    nc = bacc.Bacc(
        "TRN2", target_bir_lowering=False, debug=False, num_devices=N_CORES
    )

    q_d = nc.dram_tensor("q", [QH, Q_DIM], f32, kind="ExternalInput").ap()
    kv_d = nc.dram_tensor("kv", [NKV, KV_DIM], f32, kind="ExternalInput").ap()
    wq_d = nc.dram_tensor("wq", [ATTN_DIM, Q_DIM], f32, kind="ExternalInput").ap()
    wk_d = nc.dram_tensor("wk", [ATTN_DIM, KV_DIM], f32, kind="ExternalInput").ap()
    v_d = nc.dram_tensor("v", [1, ATTN_DIM], f32, kind="ExternalInput").ap()
    wv_d = nc.dram_tensor("wv", [Q_DIM, KV_DIM], f32, kind="ExternalInput").ap()
    wo_d = nc.dram_tensor("wo", [Q_DIM, Q_DIM], f32, kind="ExternalInput").ap()
    bo_d = nc.dram_tensor("bo", [1, Q_DIM], f32, kind="ExternalInput").ap()
    alpha_d = nc.dram_tensor("alpha", [QH, NKV], f32, kind="ExternalOutput").ap()
    out_d = nc.dram_tensor("out", [QH, Q_DIM], f32, kind="ExternalOutput").ap()

    CKV = KV_DIM // 128  # 6 c-chunks of kv dim
    CQ = Q_DIM // 128  # 4 c-chunks of q dim
    DB = ATTN_DIM // 128  # 2 d-blocks
    KC = NKV // 128  # 8 k-chunks
    MC = Q_DIM // 128  # 4 m-chunks of context dim

    with tile.TileContext(nc) as tc, ExitStack() as ctx:
        const = ctx.enter_context(tc.tile_pool(name="const", bufs=1))
        persist = ctx.enter_context(tc.tile_pool(name="persist", bufs=1))
        stage = ctx.enter_context(tc.tile_pool(name="stage", bufs=21))
        tpool = ctx.enter_context(tc.tile_pool(name="tanh", bufs=3))
        small = ctx.enter_context(tc.tile_pool(name="small", bufs=1))
        ps_kpt = ctx.enter_context(tc.tile_pool(name="ps_kpt", bufs=1, space="PSUM"))
        ps_et = ctx.enter_context(tc.tile_pool(name="ps_et", bufs=1, space="PSUM"))
        ps_tp = ctx.enter_context(tc.tile_pool(name="ps_tp", bufs=2, space="PSUM"))

        ident = const.tile([128, 128], f32, name="ident", tag="ident")
        masks.make_identity(nc, ident[:])

        # ---- all input DMAs up front, kv (the gating tensor) first ----
        kv_st = [
            stage.tile([128, KV_DIM], f32, name="stage", tag="stage") for _ in range(8)
        ]
        for rc in range(8):
            nc.sync.dma_start(kv_st[rc][:], kv_d[rc * 128 : (rc + 1) * 128, :])
        wk_st = [
            stage.tile([128, KV_DIM], f32, name="stage", tag="stage") for _ in range(DB)
        ]
        for rc in range(DB):
            nc.sync.dma_start(wk_st[rc][:], wk_d[rc * 128 : (rc + 1) * 128, :])
        q_st = stage.tile([QH, Q_DIM], f32, name="stage", tag="stage")
        nc.sync.dma_start(q_st[:], q_d[:])
        wq_st = [
            stage.tile([128, Q_DIM], f32, name="stage", tag="stage") for _ in range(DB)
        ]
        for rc in range(DB):
            nc.sync.dma_start(wq_st[rc][:], wq_d[rc * 128 : (rc + 1) * 128, :])

        v_sb = const.tile([128, DB], f32, name="v_sb", tag="v_sb")
        for db in range(DB):
            nc.sync.dma_start(
                v_sb[:, db : db + 1],
                v_d[0:1, db * 128 : (db + 1) * 128].rearrange("a b -> b a"),
            )
        v_bf = const.tile([128, DB], bf16, name="v_bf", tag="v_bf")
        nc.vector.tensor_copy(v_bf[:], v_sb[:])
        bo_sb = const.tile([1, Q_DIM], f32, name="bo_sb", tag="bo_sb")
        nc.sync.dma_start(bo_sb[:], bo_d[:])
        bo_bf = const.tile([1, Q_DIM], bf16, name="bo_bf", tag="bo_bf")
        nc.vector.tensor_copy(bo_bf[:], bo_sb[:])
        ones_sb = const.tile([1, QH], bf16, name="ones_sb", tag="ones_sb")
        nc.vector.memset(ones_sb[:], 1.0)

        def pe_t(out_ap, in_ap):
            # out = in^T via PE; in_ap [P, M] sbuf -> out_ap [M, P] psum
            kdim = in_ap.shape[0]
            nc.tensor.transpose(out_ap, in_ap, ident[0:kdim, 0:kdim])

        # ---- wk transpose: wkT[cc] [128, 256] bf16 ----
        wkT = [
            persist.tile([128, ATTN_DIM], bf16, name=f"wkT{c}", tag=f"wkT{c}")
            for c in range(CKV)
        ]
        for cc in range(CKV):
            tp = ps_tp.tile([128, 512], f32, name="tp", tag="tp")
            for j in range(DB):
                pe_t(
                    tp[:, j * 128 : (j + 1) * 128],
                    wk_st[j][:, cc * 128 : (cc + 1) * 128],
                )
            nc.vector.tensor_copy(wkT[cc][:], tp[:, 0:ATTN_DIM])

        # ---- q transpose -> qT [128, (4cc x 64)] bf16 ----
        qT = persist.tile([128, CQ * QH], bf16, name="qT", tag="qT")
        tp = ps_tp.tile([128, 512], f32, name="tp", tag="tp")
        for cc in range(CQ):
            pe_t(tp[:, cc * QH : (cc + 1) * QH], q_st[:, cc * 128 : (cc + 1) * 128])
        nc.vector.tensor_copy(qT[:], tp[:, 0 : CQ * QH])

        # ---- wq transpose: wqT[cc] [128, 256] bf16 ----
        wqT = [
            persist.tile([128, ATTN_DIM], bf16, name=f"wqT{c}", tag=f"wqT{c}")
            for c in range(CQ)
        ]
        for cc in range(CQ):
            tp = ps_tp.tile([128, 512], f32, name="tp", tag="tp")
            for j in range(DB):
                pe_t(
                    tp[:, j * 128 : (j + 1) * 128],
                    wq_st[j][:, cc * 128 : (cc + 1) * 128],
                )
            nc.vector.tensor_copy(wqT[cc][:], tp[:, 0:ATTN_DIM])

        # ---- qpT = W_q @ q^T -> [128, (db x 64)] sbuf f32 (ACT bias source) ----
        qpt_ps = ps_tp.tile([128, DB * QH], f32, name="tp", tag="tp")
        for db in range(DB):
            for cc in range(CQ):
                nc.tensor.matmul(
                    qpt_ps[:, db * QH : (db + 1) * QH],
                    lhsT=wqT[cc][:, db * 128 : (db + 1) * 128],
                    rhs=qT[:, cc * QH : (cc + 1) * QH],
                    start=(cc == 0),
                    stop=(cc == CQ - 1),
                )
        qpt = persist.tile([128, DB * QH], f32, name="qpt", tag="qpt")
        nc.vector.tensor_copy(qpt[:], qpt_ps[:])

        # ---- kv transpose: kvT[cc] [128, 1024] bf16 ----
        kvT = [
            persist.tile([128, NKV], bf16, name=f"kvT{c}", tag=f"kvT{c}")
            for c in range(CKV)
        ]
        for cc in range(CKV):
            for g in range(2):  # groups of 4 row-chunks
                tp = ps_tp.tile([128, 512], f32, name="tp", tag="tp")
                for j in range(4):
                    rc = g * 4 + j
                    pe_t(
                        tp[:, j * 128 : (j + 1) * 128],
                        kv_st[rc][:, cc * 128 : (cc + 1) * 128],
                    )
                nc.scalar.copy(kvT[cc][:, g * 512 : (g + 1) * 512], tp[:])

        # ---- kpT = W_k @ kv^T -> PSUM [128,1024] x2 d-blocks (resident) ----
        kpt = [
            ps_kpt.tile([128, NKV], f32, name=f"kpt{d}", tag=f"kpt{d}")
            for d in range(DB)
        ]
        for db in range(DB):
            for nh in range(2):
                for cc in range(CKV):
                    nc.tensor.matmul(
                        kpt[db][:, nh * 512 : (nh + 1) * 512],
                        lhsT=wkT[cc][:, db * 128 : (db + 1) * 128],
                        rhs=kvT[cc][:, nh * 512 : (nh + 1) * 512],
                        start=(cc == 0),
                        stop=(cc == CKV - 1),
                    )

        # ---- wv transpose: wvT[cc] [128, 512] bf16 ----
        wv_st = [
            stage.tile([128, KV_DIM], f32, name="stage", tag="stage") for _ in range(4)
        ]
        for rc in range(4):
            nc.sync.dma_start(wv_st[rc][:], wv_d[rc * 128 : (rc + 1) * 128, :])
        wvT = [
            persist.tile([128, Q_DIM], bf16, name=f"wvT{c}", tag=f"wvT{c}")
            for c in range(CKV)
        ]
        for cc in range(CKV):
            tp = ps_tp.tile([128, 512], f32, name="tp", tag="tp")
            for j in range(4):
                pe_t(
                    tp[:, j * 128 : (j + 1) * 128],
                    wv_st[j][:, cc * 128 : (cc + 1) * 128],
                )
            nc.vector.tensor_copy(wvT[cc][:], tp[:])

        # ---- wo transpose: woT[mc] [128, 512] bf16 ----
        wo_st = [
            stage.tile([128, Q_DIM], f32, name="stage", tag="stage") for _ in range(4)
        ]
        for rc in range(4):
            nc.sync.dma_start(wo_st[rc][:], wo_d[rc * 128 : (rc + 1) * 128, :])
        woT = [
            persist.tile([128, Q_DIM], bf16, name=f"woT{c}", tag=f"woT{c}")
            for c in range(MC)
        ]
        for cc in range(MC):
            tp = ps_tp.tile([128, 512], f32, name="tp", tag="tp")
            for j in range(4):
                pe_t(
                    tp[:, j * 128 : (j + 1) * 128],
                    wo_st[j][:, cc * 128 : (cc + 1) * 128],
                )
            nc.vector.tensor_copy(woT[cc][:], tp[:])

        # ---- v_proj = kv @ W_v^T -> vproj[kc] [128, 512] bf16 sbuf ----
        vproj = [
            persist.tile([128, Q_DIM], bf16, name=f"vproj{k}", tag=f"vproj{k}")
            for k in range(KC)
        ]
        for kc in range(KC):
            vp = ps_tp.tile([128, 512], f32, name="tp", tag="tp")
            for cc in range(CKV):
                nc.tensor.matmul(
                    vp[:],
                    lhsT=kvT[cc][:, kc * 128 : (kc + 1) * 128],
                    rhs=wvT[cc][:],
                    start=(cc == 0),
                    stop=(cc == CKV - 1),
                )
            nc.vector.tensor_copy(vproj[kc][:], vp[:])

        # ---- main loop in two halves of 32 queries, post overlapped ----
        AFt = AF.Tanh

        def q_half_loop(h):
            # energy^T for this half: [128 k-part, (kc x 32 q)] one PSUM bank
            et = ps_et.tile([128, KC * HH], f32, name=f"et{h}", tag=f"et{h}")
            for qq in range(HH):
                qi = h * HH + qq
                tt = []
                for db in range(DB):
                    t = tpool.tile([128, NKV], bf16, name=f"t{db}", tag=f"t{db}")
                    nc.scalar.activation(
                        t[:],
                        kpt[db][:],
                        AFt,
                        bias=qpt[:, db * QH + qi : db * QH + qi + 1],
                    )
                    tt.append(t)
                for kc in range(KC):
                    col = et[:, kc * HH + qq : kc * HH + qq + 1]
                    for db in range(DB):
                        nc.tensor.matmul(
                            col,
                            lhsT=tt[db][:, kc * 128 : (kc + 1) * 128],
                            rhs=v_bf[:, db : db + 1],
                            start=(db == 0),
                            stop=(db == DB - 1),
                        )
            return et

        def post_half(h, et):
            # transpose energy^T -> energy [32, 1024] (two PSUM halves)
            etsb = persist.tile([128, KC * HH], f32, name=f"etsb{h}", tag=f"etsb{h}")
            nc.vector.tensor_copy(etsb[:], et[:])
            e_ps = []
            for half in range(2):
                ep = ps_tp.tile([HH, 512], f32, name="tp", tag="tp")
                for j in range(4):
                    kc = half * 4 + j
                    pe_t(
                        ep[:, j * 128 : (j + 1) * 128],
                        etsb[:, kc * HH : (kc + 1) * HH],
                    )
                e_ps.append(ep)

            # softmax over k (free axis)
            rmax = small.tile([HH, 2], f32, name=f"rmax{h}", tag=f"rmax{h}")
            for half in range(2):
                nc.vector.tensor_reduce(
                    rmax[:, half : half + 1],
                    e_ps[half][:],
                    axis=mybir.AxisListType.X,
                    op=mybir.AluOpType.max,
                )
            nm = small.tile([HH, 1], f32, name=f"nm{h}", tag=f"nm{h}")
            nc.vector.tensor_reduce(
                nm[:], rmax[:], axis=mybir.AxisListType.X, op=mybir.AluOpType.max
            )
            nm2 = small.tile([HH, 1], f32, name=f"nm2{h}", tag=f"nm2{h}")
            nc.vector.tensor_scalar_mul(nm2[:], nm[:], -1.0)

            p_sb = persist.tile([HH, NKV], f32, name=f"p_sb{h}", tag=f"p_sb{h}")
            ssum = small.tile([HH, 2], f32, name=f"ssum{h}", tag=f"ssum{h}")
            for half in range(2):
                nc.scalar.activation(
                    p_sb[:, half * 512 : (half + 1) * 512],
                    e_ps[half][:],
                    AF.Exp,
                    bias=nm2[:],
                    accum_out=ssum[:, half : half + 1],
                )
            stot = small.tile([HH, 1], f32, name=f"stot{h}", tag=f"stot{h}")
            nc.vector.tensor_reduce(
                stot[:], ssum[:], axis=mybir.AxisListType.X, op=mybir.AluOpType.add
            )
            rinv = small.tile([HH, 1], f32, name=f"rinv{h}", tag=f"rinv{h}")
            nc.vector.reciprocal(rinv[:], stot[:])
            alpha_sb = persist.tile(
                [HH, NKV], f32, name=f"alpha_sb{h}", tag=f"alpha_sb{h}"
            )
            nc.vector.tensor_scalar_mul(alpha_sb[:], p_sb[:], rinv[:])
            nc.sync.dma_start(alpha_d[h * HH : (h + 1) * HH, :], alpha_sb[:])

            # alpha^T -> context = alpha @ v_proj
            a_ps = ps_tp.tile([128, KC * HH], f32, name="tp", tag="tp")
            for kc in range(KC):
                pe_t(
                    a_ps[:, kc * HH : (kc + 1) * HH],
                    alpha_sb[:, kc * 128 : (kc + 1) * 128],
                )
            aT = persist.tile([128, KC * HH], bf16, name=f"aT{h}", tag=f"aT{h}")
            nc.vector.tensor_copy(aT[:], a_ps[:])

            ctx_ps = ps_tp.tile([HH, Q_DIM], f32, name="tp", tag="tp")
            for kc in range(KC):
                nc.tensor.matmul(
                    ctx_ps[:],
                    lhsT=aT[:, kc * HH : (kc + 1) * HH],
                    rhs=vproj[kc][:],
                    start=(kc == 0),
                    stop=(kc == KC - 1),
                )
            ctx_sb = persist.tile([HH, Q_DIM], f32, name=f"ctx_sb{h}", tag=f"ctx_sb{h}")
            nc.vector.tensor_copy(ctx_sb[:], ctx_ps[:])

            # context^T -> out = context @ W_o^T + b_o
            c_ps = ps_tp.tile([128, MC * HH], f32, name="tp", tag="tp")
            for mc in range(MC):
                pe_t(
                    c_ps[:, mc * HH : (mc + 1) * HH],
                    ctx_sb[:, mc * 128 : (mc + 1) * 128],
                )
            cT = persist.tile([128, MC * HH], bf16, name=f"cT{h}", tag=f"cT{h}")
            nc.vector.tensor_copy(cT[:], c_ps[:])

            o_ps = ps_tp.tile([HH, Q_DIM], f32, name="tp", tag="tp")
            for mc in range(MC):
                nc.tensor.matmul(
                    o_ps[:],
                    lhsT=cT[:, mc * HH : (mc + 1) * HH],
                    rhs=woT[mc][:],
                    start=(mc == 0),
                    stop=False,
                )
            nc.tensor.matmul(
                o_ps[:], lhsT=ones_sb[:, 0:HH], rhs=bo_bf[:], start=False, stop=True
            )
            o_sb = persist.tile([HH, Q_DIM], f32, name=f"o_sb{h}", tag=f"o_sb{h}")
            nc.vector.tensor_copy(o_sb[:], o_ps[:])
            nc.sync.dma_start(out_d[h * HH : (h + 1) * HH, :], o_sb[:])

        et0 = q_half_loop(0)
        post_half(0, et0)
        et1 = q_half_loop(1)
        post_half(1, et1)

    nc.compile()
    return nc


def _get_nc():
    if "nc" not in _CACHE:
        _CACHE["nc"] = _build()
    return _CACHE["nc"]


def kernel(q, kv, W_q, W_k, v, W_v, W_o, b_o):
    global last_results
    from concourse.bass_utils import run_bass_kernel_spmd

    nc = _get_nc()
    q = np.asarray(q, dtype=np.float32)
    kv = np.asarray(kv, dtype=np.float32)
    common = {
        "wq": np.ascontiguousarray(np.asarray(W_q, dtype=np.float32)),
        "wk": np.ascontiguousarray(np.asarray(W_k, dtype=np.float32)),
        "v": np.ascontiguousarray(np.asarray(v, dtype=np.float32).reshape(1, ATTN_DIM)),
        "wv": np.ascontiguousarray(np.asarray(W_v, dtype=np.float32)),
        "wo": np.ascontiguousarray(np.asarray(W_o, dtype=np.float32)),
        "bo": np.ascontiguousarray(np.asarray(b_o, dtype=np.float32).reshape(1, Q_DIM)),
    }
    in_maps = []
    for core in range(N_CORES):
        b, h = core // 2, core % 2
        in_maps.append(
            {
                "q": np.ascontiguousarray(q[b, h * QH : (h + 1) * QH, :]),
                "kv": np.ascontiguousarray(kv[b]),
                **common,
            }
        )

    trace = bool(os.environ.get("BASS_TRACE"))
    last_results = run_bass_kernel_spmd(
        nc, in_maps, core_ids=list(range(N_CORES)), trace=trace
    )
    res = last_results.results

    out = np.empty((B, NQ, Q_DIM), dtype=np.float32)
    alpha = np.empty((B, NQ, NKV), dtype=np.float32)
    for core in range(N_CORES):
        b, h = core // 2, core % 2
        out[b, h * QH : (h + 1) * QH, :] = res[core]["out"]
        alpha[b, h * QH : (h + 1) * QH, :] = res[core]["alpha"]
    return out, alpha


# revision 7
# speedup vs baseline: 1.0251x; 1.0251x over previous
"""Bahdanau cross-attention on 8 Trainium2 NeuronCores.

Sharding: 8 cores = B(4) x NQ-halves(2). Each core owns one batch b and 64
queries, with the FULL NKV=1024 keys -> softmax is fully local, no
collectives. Host only slices inputs / concatenates outputs.

Per-core algorithm:
  - transpose kv/weights on device via PE (identity matmul), f32 data,
    PSUM->SBUF evacuation casts to bf16 for all TensorE operands
  - kpT[d=256, k=1024] = W_k @ kv^T  (PSUM f32, resident; ACT reads PSUM)
  - qpT[d=256, q=64]   = W_q @ q^T   (SBUF f32, per-partition ACT bias)
  - for each q: t[d,k] = tanh(kpT + qpT[:,q]) one ACT instr per d-block
    (bias does the broadcast add for free), t written as bf16; PE reduces
    sum_d v_d*t with t-as-stationary (bf16 LDWEIGHTS at 1 cyc/row),
    v-as-moving -> energy^T column in PSUM
  - q-loop split in two 32-query halves; each half's softmax (DVE max, ACT
    exp with bias=-max + accum_out denominator), alpha, context and output
    projection overlap the other half's tanh stream
  - out = context @ W_o^T + b_o via a ones-row matmul trick for the bias
"""

import os
import sys
from contextlib import ExitStack

import numpy as np

os.environ.setdefault("MYCRO_LOCAL_CACHE", "1")
for _p in ("/opt/trn_rl_repo", "/root/.axon_site/_ro/trn_rl_repo"):
    if os.path.isdir(_p) and _p not in sys.path:
        sys.path.append(_p)

B, NQ, NKV = 4, 128, 1024
Q_DIM, KV_DIM, ATTN_DIM = 512, 768, 256
QH = NQ // 2  # 64 queries per core
HH = QH // 2  # 32 queries per post-processing half
N_CORES = 8

_CACHE = {}
last_results = None  # BassKernelResults from the most recent run (for test.py)


def _build():
    from concourse import bacc, masks, mybir, tile

    dt = mybir.dt
    f32 = dt.float32
    bf16 = dt.bfloat16
    AF = mybir.ActivationFunctionType

    nc = bacc.Bacc(
        "TRN2", target_bir_lowering=False, debug=False, num_devices=N_CORES
    )

    q_d = nc.dram_tensor("q", [QH, Q_DIM], f32, kind="ExternalInput").ap()
    kv_d = nc.dram_tensor("kv", [NKV, KV_DIM], f32, kind="ExternalInput").ap()
    wq_d = nc.dram_tensor("wq", [ATTN_DIM, Q_DIM], f32, kind="ExternalInput").ap()
    wk_d = nc.dram_tensor("wk", [ATTN_DIM, KV_DIM], f32, kind="ExternalInput").ap()
    v_d = nc.dram_tensor("v", [1, ATTN_DIM], f32, kind="ExternalInput").ap()
    wv_d = nc.dram_tensor("wv", [Q_DIM, KV_DIM], f32, kind="ExternalInput").ap()
    wo_d = nc.dram_tensor("wo", [Q_DIM, Q_DIM], f32, kind="ExternalInput").ap()
    bo_d = nc.dram_tensor("bo", [1, Q_DIM], f32, kind="ExternalInput").ap()
    alpha_d = nc.dram_tensor("alpha", [QH, NKV], f32, kind="ExternalOutput").ap()
    out_d = nc.dram_tensor("out", [QH, Q_DIM], f32, kind="ExternalOutput").ap()

    CKV = KV_DIM // 128  # 6 c-chunks of kv dim
    CQ = Q_DIM // 128  # 4 c-chunks of q dim
    DB = ATTN_DIM // 128  # 2 d-blocks
    KC = NKV // 128  # 8 k-chunks
    MC = Q_DIM // 128  # 4 m-chunks of context dim

    with tile.TileContext(nc) as tc, ExitStack() as ctx:
        const = ctx.enter_context(tc.tile_pool(name="const", bufs=1))
        persist = ctx.enter_context(tc.tile_pool(name="persist", bufs=1))
        stage = ctx.enter_context(tc.tile_pool(name="stage", bufs=21))
        tpool = ctx.enter_context(tc.tile_pool(name="tanh", bufs=6))
        small = ctx.enter_context(tc.tile_pool(name="small", bufs=1))
        ps_kpt = ctx.enter_context(tc.tile_pool(name="ps_kpt", bufs=1, space="PSUM"))
        ps_et = ctx.enter_context(tc.tile_pool(name="ps_et", bufs=1, space="PSUM"))
        ps_tp = ctx.enter_context(tc.tile_pool(name="ps_tp", bufs=2, space="PSUM"))

        ident = const.tile([128, 128], f32, name="ident", tag="ident")
        masks.make_identity(nc, ident[:])

        # ---- all input DMAs up front, kv (the gating tensor) first ----
        kv_st = [
            stage.tile([128, KV_DIM], f32, name="stage", tag="stage") for _ in range(8)
        ]
        wk_st = [
            stage.tile([128, KV_DIM], f32, name="stage", tag="stage") for _ in range(DB)
        ]
        for rc in range(DB):
            nc.sync.dma_start(wk_st[rc][:], wk_d[rc * 128 : (rc + 1) * 128, :])
        q_st = stage.tile([QH, Q_DIM], f32, name="stage", tag="stage")
        nc.sync.dma_start(q_st[:], q_d[:])
        wq_st = [
            stage.tile([128, Q_DIM], f32, name="stage", tag="stage") for _ in range(DB)
        ]
        for rc in range(DB):
            nc.sync.dma_start(wq_st[rc][:], wq_d[rc * 128 : (rc + 1) * 128, :])

        # kv arrives column-chunk-major so transposes + k_proj pipeline with it
        for cc in range(CKV):
            for rc in range(8):
                nc.sync.dma_start(
                    kv_st[rc][:, cc * 128 : (cc + 1) * 128],
                    kv_d[rc * 128 : (rc + 1) * 128, cc * 128 : (cc + 1) * 128],
                )
        wv_st = [
            stage.tile([128, KV_DIM], f32, name="stage", tag="stage") for _ in range(4)
        ]
        for rc in range(4):
            nc.sync.dma_start(wv_st[rc][:], wv_d[rc * 128 : (rc + 1) * 128, :])

        v_sb = const.tile([128, DB], f32, name="v_sb", tag="v_sb")
        for db in range(DB):
            nc.sync.dma_start(
                v_sb[:, db : db + 1],
                v_d[0:1, db * 128 : (db + 1) * 128].rearrange("a b -> b a"),
            )
        v_bf = const.tile([128, DB], bf16, name="v_bf", tag="v_bf")
        nc.vector.tensor_copy(v_bf[:], v_sb[:])
        bo_sb = const.tile([1, Q_DIM], f32, name="bo_sb", tag="bo_sb")
        nc.sync.dma_start(bo_sb[:], bo_d[:])
        bo_bf = const.tile([1, Q_DIM], bf16, name="bo_bf", tag="bo_bf")
        nc.vector.tensor_copy(bo_bf[:], bo_sb[:])
        ones_sb = const.tile([1, QH], bf16, name="ones_sb", tag="ones_sb")
        nc.vector.memset(ones_sb[:], 1.0)

        def pe_t(out_ap, in_ap):
            # out = in^T via PE; in_ap [P, M] sbuf -> out_ap [M, P] psum
            kdim = in_ap.shape[0]
            nc.tensor.transpose(out_ap, in_ap, ident[0:kdim, 0:kdim])

        # ---- wk transpose: wkT[cc] [128, 256] bf16 ----
        wkT = [
            persist.tile([128, ATTN_DIM], bf16, name=f"wkT{c}", tag=f"wkT{c}")
            for c in range(CKV)
        ]
        for cc in range(CKV):
            tp = ps_tp.tile([128, 512], f32, name="tp", tag="tp")
            for j in range(DB):
                pe_t(
                    tp[:, j * 128 : (j + 1) * 128],
                    wk_st[j][:, cc * 128 : (cc + 1) * 128],
                )
            nc.vector.tensor_copy(wkT[cc][:], tp[:, 0:ATTN_DIM])

        # ---- q transpose -> qT [128, (4cc x 64)] bf16 ----
        qT = persist.tile([128, CQ * QH], bf16, name="qT", tag="qT")
        tp = ps_tp.tile([128, 512], f32, name="tp", tag="tp")
        for cc in range(CQ):
            pe_t(tp[:, cc * QH : (cc + 1) * QH], q_st[:, cc * 128 : (cc + 1) * 128])
        nc.vector.tensor_copy(qT[:], tp[:, 0 : CQ * QH])

        # ---- wq transpose: wqT[cc] [128, 256] bf16 ----
        wqT = [
            persist.tile([128, ATTN_DIM], bf16, name=f"wqT{c}", tag=f"wqT{c}")
            for c in range(CQ)
        ]
        for cc in range(CQ):
            tp = ps_tp.tile([128, 512], f32, name="tp", tag="tp")
            for j in range(DB):
                pe_t(
                    tp[:, j * 128 : (j + 1) * 128],
                    wq_st[j][:, cc * 128 : (cc + 1) * 128],
                )
            nc.vector.tensor_copy(wqT[cc][:], tp[:, 0:ATTN_DIM])

        # ---- qpT = W_q @ q^T -> [128, (db x 64)] sbuf f32 (ACT bias source) ----
        qpt_ps = ps_tp.tile([128, DB * QH], f32, name="tp", tag="tp")
        for db in range(DB):
            for cc in range(CQ):
                nc.tensor.matmul(
                    qpt_ps[:, db * QH : (db + 1) * QH],
                    lhsT=wqT[cc][:, db * 128 : (db + 1) * 128],
                    rhs=qT[:, cc * QH : (cc + 1) * QH],
                    start=(cc == 0),
                    stop=(cc == CQ - 1),
                )
        qpt = persist.tile([128, DB * QH], f32, name="qpt", tag="qpt")
        nc.vector.tensor_copy(qpt[:], qpt_ps[:])

        # ---- kv transpose + k_proj interleaved per c-chunk ----
        kvT = [
            persist.tile([128, NKV], bf16, name=f"kvT{c}", tag=f"kvT{c}")
            for c in range(CKV)
        ]
        kpt = [
            ps_kpt.tile([128, NKV], f32, name=f"kpt{d}", tag=f"kpt{d}")
            for d in range(DB)
        ]
        for cc in range(CKV):
            for g in range(2):  # groups of 4 row-chunks
                tp = ps_tp.tile([128, 512], f32, name="tp", tag="tp")
                for j in range(4):
                    rc = g * 4 + j
                    pe_t(
                        tp[:, j * 128 : (j + 1) * 128],
                        kv_st[rc][:, cc * 128 : (cc + 1) * 128],
                    )
                nc.scalar.copy(kvT[cc][:, g * 512 : (g + 1) * 512], tp[:])
            for db in range(DB):
                for nh in range(2):
                    nc.tensor.matmul(
                        kpt[db][:, nh * 512 : (nh + 1) * 512],
                        lhsT=wkT[cc][:, db * 128 : (db + 1) * 128],
                        rhs=kvT[cc][:, nh * 512 : (nh + 1) * 512],
                        start=(cc == 0),
                        stop=(cc == CKV - 1),
                        skip_group_check=True,
                    )

        # ---- wv transpose: wvT[cc] [128, 512] bf16 ----
        wvT = [
            persist.tile([128, Q_DIM], bf16, name=f"wvT{c}", tag=f"wvT{c}")
            for c in range(CKV)
        ]
        for cc in range(CKV):
            tp = ps_tp.tile([128, 512], f32, name="tp", tag="tp")
            for j in range(4):
                pe_t(
                    tp[:, j * 128 : (j + 1) * 128],
                    wv_st[j][:, cc * 128 : (cc + 1) * 128],
                )
            nc.vector.tensor_copy(wvT[cc][:], tp[:])

        # ---- v_proj = kv @ W_v^T -> vproj[kc] [128, 512] bf16 sbuf ----
        vproj = [
            persist.tile([128, Q_DIM], bf16, name=f"vproj{k}", tag=f"vproj{k}")
            for k in range(KC)
        ]
        for kc in range(KC):
            vp = ps_tp.tile([128, 512], f32, name="tp", tag="tp")
            for cc in range(CKV):
                nc.tensor.matmul(
                    vp[:],
                    lhsT=kvT[cc][:, kc * 128 : (kc + 1) * 128],
                    rhs=wvT[cc][:],
                    start=(cc == 0),
                    stop=(cc == CKV - 1),
                )
            nc.vector.tensor_copy(vproj[kc][:], vp[:])

        # ---- main loop in two halves of 32 queries, post overlapped ----
        AFt = AF.Tanh

        def q_half_part(h, qq0, qq1, et=None):
            # energy^T for this half: [128 k-part, (kc x 32 q)] one PSUM bank
            if et is None:
                et = ps_et.tile([128, KC * HH], f32, name=f"et{h}", tag=f"et{h}")
            for qq in range(qq0, qq1):
                qi = h * HH + qq
                tt = []
                for db in range(DB):
                    t = tpool.tile([128, NKV], bf16, name=f"t{db}", tag=f"t{db}")
                    nc.scalar.activation(
                        t[:],
                        kpt[db][:],
                        AFt,
                        bias=qpt[:, db * QH + qi : db * QH + qi + 1],
                    )
                    tt.append(t)
                for kc in range(KC):
                    col = et[:, kc * HH + qq : kc * HH + qq + 1]
                    for db in range(DB):
                        nc.tensor.matmul(
                            col,
                            lhsT=tt[db][:, kc * 128 : (kc + 1) * 128],
                            rhs=v_bf[:, db : db + 1],
                            start=(db == 0),
                            stop=(db == DB - 1),
                        )
            return et

        def q_half_loop(h):
            return q_half_part(h, 0, HH)

        def post_half(h, et):
            # transpose energy^T -> energy [32, 1024] (two PSUM halves)
            etsb = persist.tile([128, KC * HH], f32, name=f"etsb{h}", tag=f"etsb{h}")
            nc.vector.tensor_copy(etsb[:], et[:])
            e_ps = []
            for half in range(2):
                ep = ps_tp.tile([HH, 512], f32, name="tp", tag="tp")
                for j in range(4):
                    kc = half * 4 + j
                    pe_t(
                        ep[:, j * 128 : (j + 1) * 128],
                        etsb[:, kc * HH : (kc + 1) * HH],
                    )
                e_ps.append(ep)

            # softmax over k (free axis)
            rmax = small.tile([HH, 2], f32, name=f"rmax{h}", tag=f"rmax{h}")
            for half in range(2):
                nc.vector.tensor_reduce(
                    rmax[:, half : half + 1],
                    e_ps[half][:],
                    axis=mybir.AxisListType.X,
                    op=mybir.AluOpType.max,
                )
            nm = small.tile([HH, 1], f32, name=f"nm{h}", tag=f"nm{h}")
            nc.vector.tensor_reduce(
                nm[:], rmax[:], axis=mybir.AxisListType.X, op=mybir.AluOpType.max
            )
            nm2 = small.tile([HH, 1], f32, name=f"nm2{h}", tag=f"nm2{h}")
            nc.vector.tensor_scalar_mul(nm2[:], nm[:], -1.0)

            p_sb = persist.tile([HH, NKV], f32, name=f"p_sb{h}", tag=f"p_sb{h}")
            ssum = small.tile([HH, 2], f32, name=f"ssum{h}", tag=f"ssum{h}")
            for half in range(2):
                nc.scalar.activation(
                    p_sb[:, half * 512 : (half + 1) * 512],
                    e_ps[half][:],
                    AF.Exp,
                    bias=nm2[:],
                    accum_out=ssum[:, half : half + 1],
                )
            stot = small.tile([HH, 1], f32, name=f"stot{h}", tag=f"stot{h}")
            nc.vector.tensor_reduce(
                stot[:], ssum[:], axis=mybir.AxisListType.X, op=mybir.AluOpType.add
            )
            rinv = small.tile([HH, 1], f32, name=f"rinv{h}", tag=f"rinv{h}")
            nc.vector.reciprocal(rinv[:], stot[:])
            alpha_sb = persist.tile(
                [HH, NKV], f32, name=f"alpha_sb{h}", tag=f"alpha_sb{h}"
            )
            nc.vector.tensor_scalar_mul(alpha_sb[:], p_sb[:], rinv[:])
            nc.sync.dma_start(alpha_d[h * HH : (h + 1) * HH, :], alpha_sb[:])

            # alpha^T -> context = alpha @ v_proj
            a_ps = ps_tp.tile([128, KC * HH], f32, name="tp", tag="tp")
            for kc in range(KC):
                pe_t(
                    a_ps[:, kc * HH : (kc + 1) * HH],
                    alpha_sb[:, kc * 128 : (kc + 1) * 128],
                )
            aT = persist.tile([128, KC * HH], bf16, name=f"aT{h}", tag=f"aT{h}")
            nc.vector.tensor_copy(aT[:], a_ps[:])

            ctx_ps = ps_tp.tile([HH, Q_DIM], f32, name="tp", tag="tp")
            for kc in range(KC):
                nc.tensor.matmul(
                    ctx_ps[:],
                    lhsT=aT[:, kc * HH : (kc + 1) * HH],
                    rhs=vproj[kc][:],
                    start=(kc == 0),
                    stop=(kc == KC - 1),
                )
            ctx_sb = persist.tile([HH, Q_DIM], f32, name=f"ctx_sb{h}", tag=f"ctx_sb{h}")
            nc.vector.tensor_copy(ctx_sb[:], ctx_ps[:])

            # context^T -> out = context @ W_o^T + b_o
            c_ps = ps_tp.tile([128, MC * HH], f32, name="tp", tag="tp")
            for mc in range(MC):
                pe_t(
                    c_ps[:, mc * HH : (mc + 1) * HH],
                    ctx_sb[:, mc * 128 : (mc + 1) * 128],
                )
            cT = persist.tile([128, MC * HH], bf16, name=f"cT{h}", tag=f"cT{h}")
            nc.vector.tensor_copy(cT[:], c_ps[:])

            o_ps = ps_tp.tile([HH, Q_DIM], f32, name="tp", tag="tp")
            for mc in range(MC):
                nc.tensor.matmul(
                    o_ps[:],
                    lhsT=cT[:, mc * HH : (mc + 1) * HH],
                    rhs=woT[mc][:],
                    start=(mc == 0),
                    stop=False,
                )
            nc.tensor.matmul(
                o_ps[:], lhsT=ones_sb[:, 0:HH], rhs=bo_bf[:], start=False, stop=True
            )
            o_sb = persist.tile([HH, Q_DIM], f32, name=f"o_sb{h}", tag=f"o_sb{h}")
            nc.vector.tensor_copy(o_sb[:], o_ps[:])
            nc.sync.dma_start(out_d[h * HH : (h + 1) * HH, :], o_sb[:])

        def wo_chain():
            wo_st = [
                stage.tile([128, Q_DIM], f32, name="wo_stage", tag="wo_stage")
                for _ in range(4)
            ]
            for rc in range(4):
                nc.sync.dma_start(wo_st[rc][:], wo_d[rc * 128 : (rc + 1) * 128, :])
            woT = [
                persist.tile([128, Q_DIM], bf16, name=f"woT{c}", tag=f"woT{c}")
                for c in range(MC)
            ]
            for cc in range(MC):
                tp = ps_tp.tile([128, 512], f32, name="tp", tag="tp")
                for j in range(4):
                    pe_t(
                        tp[:, j * 128 : (j + 1) * 128],
                        wo_st[j][:, cc * 128 : (cc + 1) * 128],
                    )
                nc.vector.tensor_copy(woT[cc][:], tp[:])
            return woT

        et0 = q_half_loop(0)
        woT = wo_chain()
        et1_a = q_half_part(1, 0, 8)
        post_half(0, et0)
        q_half_part(1, 8, HH, et1_a)
        post_half(1, et1_a)

    nc.compile()
    return nc


def _get_nc():
    if "nc" not in _CACHE:
        _CACHE["nc"] = _build()
    return _CACHE["nc"]


def kernel(q, kv, W_q, W_k, v, W_v, W_o, b_o):
    global last_results
    from concourse.bass_utils import run_bass_kernel_spmd

    nc = _get_nc()
    q = np.asarray(q, dtype=np.float32)
    kv = np.asarray(kv, dtype=np.float32)
    common = {
        "wq": np.ascontiguousarray(np.asarray(W_q, dtype=np.float32)),
        "wk": np.ascontiguousarray(np.asarray(W_k, dtype=np.float32)),
        "v": np.ascontiguousarray(np.asarray(v, dtype=np.float32).reshape(1, ATTN_DIM)),
        "wv": np.ascontiguousarray(np.asarray(W_v, dtype=np.float32)),
        "wo": np.ascontiguousarray(np.asarray(W_o, dtype=np.float32)),
        "bo": np.ascontiguousarray(np.asarray(b_o, dtype=np.float32).reshape(1, Q_DIM)),
    }
    in_maps = []
    for core in range(N_CORES):
        b, h = core // 2, core % 2
        in_maps.append(
            {
                "q": np.ascontiguousarray(q[b, h * QH : (h + 1) * QH, :]),
                "kv": np.ascontiguousarray(kv[b]),
                **common,
            }
        )

    trace = bool(os.environ.get("BASS_TRACE"))
    last_results = run_bass_kernel_spmd(
        nc, in_maps, core_ids=list(range(N_CORES)), trace=trace
    )
    res = last_results.results

    out = np.empty((B, NQ, Q_DIM), dtype=np.float32)
    alpha = np.empty((B, NQ, NKV), dtype=np.float32)
    for core in range(N_CORES):
        b, h = core // 2, core % 2
        out[b, h * QH : (h + 1) * QH, :] = res[core]["out"]
        alpha[b, h * QH : (h + 1) * QH, :] = res[core]["alpha"]
    return out, alpha


# revision 10
# speedup vs baseline: 1.1216x; 1.0941x over previous
"""Bahdanau cross-attention on 8 Trainium2 NeuronCores.

Sharding: 8 cores = B(4) x NQ-halves(2). Each core owns one batch b and 64
queries, with the FULL NKV=1024 keys -> softmax is fully local, no
collectives. Host only slices inputs / concatenates outputs.

Per-core algorithm:
  - transpose kv/weights on device via PE (identity matmul), f32 data,
    PSUM->SBUF evacuation casts to bf16 for all TensorE operands
  - kpT[d=256, k=1024] = W_k @ kv^T  (PSUM f32, resident; ACT reads PSUM)
  - qpT[d=256, q=64]   = W_q @ q^T   (SBUF f32, per-partition ACT bias)
  - for each q: t[d,k] = tanh(kpT + qpT[:,q]) one ACT instr per d-block
    (bias does the broadcast add for free), t written as bf16; PE reduces
    sum_d v_d*t with t-as-stationary (bf16 LDWEIGHTS at 1 cyc/row),
    v-as-moving -> energy^T column in PSUM
  - q-loop split in two 32-query halves; each half's softmax (DVE max, ACT
    exp with bias=-max + accum_out denominator), alpha, context and output
    projection overlap the other half's tanh stream
  - out = context @ W_o^T + b_o via a ones-row matmul trick for the bias
"""

import os
import sys
from contextlib import ExitStack

import numpy as np

os.environ.setdefault("MYCRO_LOCAL_CACHE", "1")
for _p in ("/opt/trn_rl_repo", "/root/.axon_site/_ro/trn_rl_repo"):
    if os.path.isdir(_p) and _p not in sys.path:
        sys.path.append(_p)

B, NQ, NKV = 4, 128, 1024
Q_DIM, KV_DIM, ATTN_DIM = 512, 768, 256
QH = NQ // 2  # 64 queries per core
HH = QH // 2  # 32 queries per post-processing half
N_CORES = 8

_CACHE = {}
last_results = None  # BassKernelResults from the most recent run (for test.py)


def _build():
    from concourse import bacc, masks, mybir, tile

    dt = mybir.dt
    f32 = dt.float32
    bf16 = dt.bfloat16
    AF = mybir.ActivationFunctionType

    nc = bacc.Bacc(
        "TRN2", target_bir_lowering=False, debug=False, num_devices=N_CORES
    )

    q_d = nc.dram_tensor("q", [QH, Q_DIM], f32, kind="ExternalInput").ap()
    kv_d = nc.dram_tensor("kv", [NKV, KV_DIM], f32, kind="ExternalInput").ap()
    wq_d = nc.dram_tensor("wq", [ATTN_DIM, Q_DIM], f32, kind="ExternalInput").ap()
    wk_d = nc.dram_tensor("wk", [ATTN_DIM, KV_DIM], f32, kind="ExternalInput").ap()
    v_d = nc.dram_tensor("v", [1, ATTN_DIM], f32, kind="ExternalInput").ap()
    wv_d = nc.dram_tensor("wv", [Q_DIM, KV_DIM], f32, kind="ExternalInput").ap()
    wo_d = nc.dram_tensor("wo", [Q_DIM, Q_DIM], f32, kind="ExternalInput").ap()
    bo_d = nc.dram_tensor("bo", [1, Q_DIM], f32, kind="ExternalInput").ap()
    alpha_d = nc.dram_tensor("alpha", [QH, NKV], f32, kind="ExternalOutput").ap()
    out_d = nc.dram_tensor("out", [QH, Q_DIM], f32, kind="ExternalOutput").ap()

    CKV = KV_DIM // 128  # 6 c-chunks of kv dim
    CQ = Q_DIM // 128  # 4 c-chunks of q dim
    DB = ATTN_DIM // 128  # 2 d-blocks
    KC = NKV // 128  # 8 k-chunks
    MC = Q_DIM // 128  # 4 m-chunks of context dim

    with tile.TileContext(nc) as tc, ExitStack() as ctx:
        const = ctx.enter_context(tc.tile_pool(name="const", bufs=1))
        persist = ctx.enter_context(tc.tile_pool(name="persist", bufs=1))
        stage = ctx.enter_context(tc.tile_pool(name="stage", bufs=9))
        tpool = ctx.enter_context(tc.tile_pool(name="tanh", bufs=8))
        small = ctx.enter_context(tc.tile_pool(name="small", bufs=1))
        ps_kpt = ctx.enter_context(tc.tile_pool(name="ps_kpt", bufs=1, space="PSUM"))
        ps_et = ctx.enter_context(tc.tile_pool(name="ps_et", bufs=1, space="PSUM"))
        ps_tp = ctx.enter_context(tc.tile_pool(name="ps_tp", bufs=2, space="PSUM"))

        ident = const.tile([128, 128], f32, name="ident", tag="ident")
        masks.make_identity(nc, ident[:])

        # ---- all input DMAs up front, kv (the gating tensor) first ----
        kv_cc = [
            stage.tile([128, NKV], f32, name="kv_cc", tag="kv_cc", bufs=CKV)
            for _ in range(CKV)
        ]
        wk_st = [
            stage.tile([128, KV_DIM], f32, name="stage", tag="stage") for _ in range(DB)
        ]
        for rc in range(DB):
            nc.sync.dma_start(wk_st[rc][:], wk_d[rc * 128 : (rc + 1) * 128, :])
        q_st = stage.tile([QH, Q_DIM], f32, name="stage", tag="stage")
        nc.sync.dma_start(q_st[:], q_d[:])
        wq_st = [
            stage.tile([128, Q_DIM], f32, name="stage", tag="stage") for _ in range(DB)
        ]
        for rc in range(DB):
            nc.sync.dma_start(wq_st[rc][:], wq_d[rc * 128 : (rc + 1) * 128, :])

        # kv arrives column-chunk-major (one big DMA per c-chunk) so the
        # transpose + k_proj chain pipelines with arrival
        for cc in range(CKV):
            nc.sync.dma_start(
                kv_cc[cc][:].rearrange("p (rc c) -> p rc c", c=128),
                kv_d[:, cc * 128 : (cc + 1) * 128].rearrange(
                    "(rc p) c -> p rc c", p=128
                ),
            )
        wv_st = [
            stage.tile([128, KV_DIM], f32, name="stage", tag="stage") for _ in range(4)
        ]
        for rc in range(4):
            nc.sync.dma_start(wv_st[rc][:], wv_d[rc * 128 : (rc + 1) * 128, :])

        v_sb = const.tile([128, DB], f32, name="v_sb", tag="v_sb")
        for db in range(DB):
            nc.sync.dma_start(
                v_sb[:, db : db + 1],
                v_d[0:1, db * 128 : (db + 1) * 128].rearrange("a b -> b a"),
            )
        v_bf = const.tile([128, DB], bf16, name="v_bf", tag="v_bf")
        nc.vector.tensor_copy(v_bf[:], v_sb[:])
        bo_sb = const.tile([1, Q_DIM], f32, name="bo_sb", tag="bo_sb")
        nc.sync.dma_start(bo_sb[:], bo_d[:])
        bo_bf = const.tile([1, Q_DIM], bf16, name="bo_bf", tag="bo_bf")
        nc.vector.tensor_copy(bo_bf[:], bo_sb[:])
        ones_sb = const.tile([1, QH], bf16, name="ones_sb", tag="ones_sb")
        nc.vector.memset(ones_sb[:], 1.0)

        def pe_t(out_ap, in_ap):
            # out = in^T via PE; in_ap [P, M] sbuf -> out_ap [M, P] psum
            kdim = in_ap.shape[0]
            nc.tensor.transpose(out_ap, in_ap, ident[0:kdim, 0:kdim])

        # ---- wk transpose: wkT[cc] [128, 256] bf16 ----
        wkT = [
            persist.tile([128, ATTN_DIM], bf16, name=f"wkT{c}", tag=f"wkT{c}")
            for c in range(CKV)
        ]
        for cc in range(CKV):
            tp = ps_tp.tile([128, 512], f32, name="tp", tag="tp")
            for j in range(DB):
                pe_t(
                    tp[:, j * 128 : (j + 1) * 128],
                    wk_st[j][:, cc * 128 : (cc + 1) * 128],
                )
            nc.scalar.copy(wkT[cc][:], tp[:, 0:ATTN_DIM])

        # ---- q transpose -> qT [128, (4cc x 64)] bf16 ----
        qT = persist.tile([128, CQ * QH], bf16, name="qT", tag="qT")
        tp = ps_tp.tile([128, 512], f32, name="tp", tag="tp")
        for cc in range(CQ):
            pe_t(tp[:, cc * QH : (cc + 1) * QH], q_st[:, cc * 128 : (cc + 1) * 128])
        nc.vector.tensor_copy(qT[:], tp[:, 0 : CQ * QH])

        # ---- wq transpose: wqT[cc] [128, 256] bf16 ----
        wqT = [
            persist.tile([128, ATTN_DIM], bf16, name=f"wqT{c}", tag=f"wqT{c}")
            for c in range(CQ)
        ]
        for cc in range(CQ):
            tp = ps_tp.tile([128, 512], f32, name="tp", tag="tp")
            for j in range(DB):
                pe_t(
                    tp[:, j * 128 : (j + 1) * 128],
                    wq_st[j][:, cc * 128 : (cc + 1) * 128],
                )
            nc.vector.tensor_copy(wqT[cc][:], tp[:, 0:ATTN_DIM])

        # ---- qpT = W_q @ q^T -> [128, (db x 64)] sbuf f32 (ACT bias source) ----
        qpt_ps = ps_tp.tile([128, DB * QH], f32, name="tp", tag="tp")
        for db in range(DB):
            for cc in range(CQ):
                nc.tensor.matmul(
                    qpt_ps[:, db * QH : (db + 1) * QH],
                    lhsT=wqT[cc][:, db * 128 : (db + 1) * 128],
                    rhs=qT[:, cc * QH : (cc + 1) * QH],
                    start=(cc == 0),
                    stop=(cc == CQ - 1),
                )
        qpt = persist.tile([128, DB * QH], f32, name="qpt", tag="qpt")
        nc.vector.tensor_copy(qpt[:], qpt_ps[:])

        # ---- kv transpose + k_proj interleaved per c-chunk ----
        kvT = [
            persist.tile([128, NKV], bf16, name=f"kvT{c}", tag=f"kvT{c}")
            for c in range(CKV)
        ]
        kpt = [
            ps_kpt.tile([128, NKV], f32, name=f"kpt{d}", tag=f"kpt{d}")
            for d in range(DB)
        ]
        for cc in range(CKV):
            for g in range(2):  # groups of 4 row-chunks
                tp = ps_tp.tile([128, 512], f32, name="tp", tag="tp")
                for j in range(4):
                    rc = g * 4 + j
                    pe_t(
                        tp[:, j * 128 : (j + 1) * 128],
                        kv_cc[cc][:, rc * 128 : (rc + 1) * 128],
                    )
                nc.scalar.copy(kvT[cc][:, g * 512 : (g + 1) * 512], tp[:])
            for db in range(DB):
                for nh in range(2):
                    nc.tensor.matmul(
                        kpt[db][:, nh * 512 : (nh + 1) * 512],
                        lhsT=wkT[cc][:, db * 128 : (db + 1) * 128],
                        rhs=kvT[cc][:, nh * 512 : (nh + 1) * 512],
                        start=(cc == 0),
                        stop=(cc == CKV - 1),
                        skip_group_check=True,
                    )

        # ---- wv transpose: wvT[cc] [128, 512] bf16 ----
        wvT = [
            persist.tile([128, Q_DIM], bf16, name=f"wvT{c}", tag=f"wvT{c}")
            for c in range(CKV)
        ]
        for cc in range(CKV):
            tp = ps_tp.tile([128, 512], f32, name="tp", tag="tp")
            for j in range(4):
                pe_t(
                    tp[:, j * 128 : (j + 1) * 128],
                    wv_st[j][:, cc * 128 : (cc + 1) * 128],
                )
            nc.vector.tensor_copy(wvT[cc][:], tp[:])

        # ---- v_proj = kv @ W_v^T -> vproj[kc] [128, 512] bf16 sbuf ----
        # emitted one kc-chunk at a time, interleaved into the h0 tanh loop
        vproj = [
            persist.tile([128, Q_DIM], bf16, name=f"vproj{k}", tag=f"vproj{k}")
            for k in range(KC)
        ]

        def emit_vproj(kc):
            vp = ps_tp.tile([128, 512], f32, name="tp", tag="tp")
            for cc in range(CKV):
                nc.tensor.matmul(
                    vp[:],
                    lhsT=kvT[cc][:, kc * 128 : (kc + 1) * 128],
                    rhs=wvT[cc][:],
                    start=(cc == 0),
                    stop=(cc == CKV - 1),
                )
            nc.vector.tensor_copy(vproj[kc][:], vp[:])

        # ---- main loop in two halves of 32 queries, post overlapped ----
        AFt = AF.Tanh

        def q_half_part(h, qq0, qq1, et=None, filler=None):
            if et is None:
                et = ps_et.tile([128, KC * HH], f32, name=f"et{h}", tag=f"et{h}")
            for qq in range(qq0, qq1):
                qi = h * HH + qq
                tt = []
                for db in range(DB):
                    t = tpool.tile([128, NKV], bf16, name=f"t{db}", tag=f"t{db}")
                    nc.scalar.activation(
                        t[:],
                        kpt[db][:],
                        AFt,
                        bias=qpt[:, db * QH + qi : db * QH + qi + 1],
                    )
                    tt.append(t)
                for kc in range(KC):
                    col = et[:, kc * HH + qq : kc * HH + qq + 1]
                    for db in range(DB):
                        nc.tensor.matmul(
                            col,
                            lhsT=tt[db][:, kc * 128 : (kc + 1) * 128],
                            rhs=v_bf[:, db : db + 1],
                            start=(db == 0),
                            stop=(db == DB - 1),
                        )
                if filler is not None and qq % 4 == 3:
                    filler(qq // 4)
            return et

        def post_half(h, et):
            # transpose energy^T -> energy [32, 1024] (two PSUM halves)
            etsb = persist.tile([128, KC * HH], f32, name=f"etsb{h}", tag=f"etsb{h}")
            nc.vector.tensor_copy(etsb[:], et[:])
            e_ps = []
            for half in range(2):
                ep = ps_tp.tile([HH, 512], f32, name="tp", tag="tp")
                for j in range(4):
                    kc = half * 4 + j
                    pe_t(
                        ep[:, j * 128 : (j + 1) * 128],
                        etsb[:, kc * HH : (kc + 1) * HH],
                    )
                e_ps.append(ep)

            # softmax over k (free axis)
            rmax = small.tile([HH, 2], f32, name=f"rmax{h}", tag=f"rmax{h}")
            for half in range(2):
                nc.vector.tensor_reduce(
                    rmax[:, half : half + 1],
                    e_ps[half][:],
                    axis=mybir.AxisListType.X,
                    op=mybir.AluOpType.max,
                )
            nm = small.tile([HH, 1], f32, name=f"nm{h}", tag=f"nm{h}")
            nc.vector.tensor_reduce(
                nm[:], rmax[:], axis=mybir.AxisListType.X, op=mybir.AluOpType.max
            )
            nm2 = small.tile([HH, 1], f32, name=f"nm2{h}", tag=f"nm2{h}")
            nc.vector.tensor_scalar_mul(nm2[:], nm[:], -1.0)

            p_sb = persist.tile([HH, NKV], f32, name=f"p_sb{h}", tag=f"p_sb{h}")
            ssum = small.tile([HH, 2], f32, name=f"ssum{h}", tag=f"ssum{h}")
            for half in range(2):
                nc.scalar.activation(
                    p_sb[:, half * 512 : (half + 1) * 512],
                    e_ps[half][:],
                    AF.Exp,
                    bias=nm2[:],
                    accum_out=ssum[:, half : half + 1],
                )
            stot = small.tile([HH, 1], f32, name=f"stot{h}", tag=f"stot{h}")
            nc.vector.tensor_reduce(
                stot[:], ssum[:], axis=mybir.AxisListType.X, op=mybir.AluOpType.add
            )
            rinv = small.tile([HH, 1], f32, name=f"rinv{h}", tag=f"rinv{h}")
            nc.vector.reciprocal(rinv[:], stot[:])
            alpha_sb = persist.tile(
                [HH, NKV], f32, name=f"alpha_sb{h}", tag=f"alpha_sb{h}"
            )
            nc.vector.tensor_scalar_mul(alpha_sb[:], p_sb[:], rinv[:])
            nc.sync.dma_start(alpha_d[h * HH : (h + 1) * HH, :], alpha_sb[:])

            # alpha^T -> context = alpha @ v_proj
            a_ps = ps_tp.tile([128, KC * HH], f32, name="tp", tag="tp")
            for kc in range(KC):
                pe_t(
                    a_ps[:, kc * HH : (kc + 1) * HH],
                    alpha_sb[:, kc * 128 : (kc + 1) * 128],
                )
            aT = persist.tile([128, KC * HH], bf16, name=f"aT{h}", tag=f"aT{h}")
            nc.vector.tensor_copy(aT[:], a_ps[:])

            ctx_ps = ps_tp.tile([HH, Q_DIM], f32, name="tp", tag="tp")
            for kc in range(KC):
                nc.tensor.matmul(
                    ctx_ps[:],
                    lhsT=aT[:, kc * HH : (kc + 1) * HH],
                    rhs=vproj[kc][:],
                    start=(kc == 0),
                    stop=(kc == KC - 1),
                )
            ctx_sb = persist.tile([HH, Q_DIM], f32, name=f"ctx_sb{h}", tag=f"ctx_sb{h}")
            nc.vector.tensor_copy(ctx_sb[:], ctx_ps[:])

            # context^T -> out = context @ W_o^T + b_o
            c_ps = ps_tp.tile([128, MC * HH], f32, name="tp", tag="tp")
            for mc in range(MC):
                pe_t(
                    c_ps[:, mc * HH : (mc + 1) * HH],
                    ctx_sb[:, mc * 128 : (mc + 1) * 128],
                )
            cT = persist.tile([128, MC * HH], bf16, name=f"cT{h}", tag=f"cT{h}")
            nc.vector.tensor_copy(cT[:], c_ps[:])

            o_ps = ps_tp.tile([HH, Q_DIM], f32, name="tp", tag="tp")
            for mc in range(MC):
                nc.tensor.matmul(
                    o_ps[:],
                    lhsT=cT[:, mc * HH : (mc + 1) * HH],
                    rhs=woT[mc][:],
                    start=(mc == 0),
                    stop=False,
                )
            nc.tensor.matmul(
                o_ps[:], lhsT=ones_sb[:, 0:HH], rhs=bo_bf[:], start=False, stop=True
            )
            o_sb = persist.tile([HH, Q_DIM], f32, name=f"o_sb{h}", tag=f"o_sb{h}")
            nc.vector.tensor_copy(o_sb[:], o_ps[:])
            nc.sync.dma_start(out_d[h * HH : (h + 1) * HH, :], o_sb[:])

        def wo_chain():
            wo_st = [
                stage.tile([128, Q_DIM], f32, name="wo_stage", tag="wo_stage")
                for _ in range(4)
            ]
            for rc in range(4):
                nc.sync.dma_start(wo_st[rc][:], wo_d[rc * 128 : (rc + 1) * 128, :])
            woT = [
                persist.tile([128, Q_DIM], bf16, name=f"woT{c}", tag=f"woT{c}")
                for c in range(MC)
            ]
            for cc in range(MC):
                tp = ps_tp.tile([128, 512], f32, name="tp", tag="tp")
                for j in range(4):
                    pe_t(
                        tp[:, j * 128 : (j + 1) * 128],
                        wo_st[j][:, cc * 128 : (cc + 1) * 128],
                    )
                nc.vector.tensor_copy(woT[cc][:], tp[:])
            return woT

        et0 = q_half_part(0, 0, HH, filler=emit_vproj)
        woT = wo_chain()
        et1_a = q_half_part(1, 0, 8)
        post_half(0, et0)
        q_half_part(1, 8, HH, et1_a)
        post_half(1, et1_a)

    nc.compile()
    return nc


def _get_nc():
    if "nc" not in _CACHE:
        _CACHE["nc"] = _build()
    return _CACHE["nc"]


def kernel(q, kv, W_q, W_k, v, W_v, W_o, b_o):
    global last_results
    from concourse.bass_utils import run_bass_kernel_spmd

    nc = _get_nc()
    q = np.asarray(q, dtype=np.float32)
    kv = np.asarray(kv, dtype=np.float32)
    common = {
        "wq": np.ascontiguousarray(np.asarray(W_q, dtype=np.float32)),
        "wk": np.ascontiguousarray(np.asarray(W_k, dtype=np.float32)),
        "v": np.ascontiguousarray(np.asarray(v, dtype=np.float32).reshape(1, ATTN_DIM)),
        "wv": np.ascontiguousarray(np.asarray(W_v, dtype=np.float32)),
        "wo": np.ascontiguousarray(np.asarray(W_o, dtype=np.float32)),
        "bo": np.ascontiguousarray(np.asarray(b_o, dtype=np.float32).reshape(1, Q_DIM)),
    }
    in_maps = []
    for core in range(N_CORES):
        b, h = core // 2, core % 2
        in_maps.append(
            {
                "q": np.ascontiguousarray(q[b, h * QH : (h + 1) * QH, :]),
                "kv": np.ascontiguousarray(kv[b]),
                **common,
            }
        )

    trace = bool(os.environ.get("BASS_TRACE"))
    last_results = run_bass_kernel_spmd(
        nc, in_maps, core_ids=list(range(N_CORES)), trace=trace
    )
    res = last_results.results

    out = np.empty((B, NQ, Q_DIM), dtype=np.float32)
    alpha = np.empty((B, NQ, NKV), dtype=np.float32)
    for core in range(N_CORES):
        b, h = core // 2, core % 2
        out[b, h * QH : (h + 1) * QH, :] = res[core]["out"]
        alpha[b, h * QH : (h + 1) * QH, :] = res[core]["alpha"]
    return out, alpha


# revision 11
# speedup vs baseline: 1.1222x; 1.0006x over previous
"""Bahdanau cross-attention on 8 Trainium2 NeuronCores.

Sharding: 8 cores = B(4) x NQ-halves(2). Each core owns one batch b and 64
queries, with the FULL NKV=1024 keys -> softmax is fully local, no
collectives. Host only slices inputs / concatenates outputs.

Per-core algorithm:
  - transpose kv/weights on device via PE (identity matmul), f32 data,
    PSUM->SBUF evacuation casts to bf16 for all TensorE operands
  - kpT[d=256, k=1024] = W_k @ kv^T  (PSUM f32, resident; ACT reads PSUM)
  - qpT[d=256, q=64]   = W_q @ q^T   (SBUF f32, per-partition ACT bias)
  - for each q: t[d,k] = tanh(kpT + qpT[:,q]) one ACT instr per d-block
    (bias does the broadcast add for free), t written as bf16; PE reduces
    sum_d v_d*t with t-as-stationary (bf16 LDWEIGHTS at 1 cyc/row),
    v-as-moving -> energy^T column in PSUM
  - q-loop split in two 32-query halves; each half's softmax (DVE max, ACT
    exp with bias=-max + accum_out denominator), alpha, context and output
    projection overlap the other half's tanh stream
  - out = context @ W_o^T + b_o via a ones-row matmul trick for the bias
"""

import os
import sys
from contextlib import ExitStack

import numpy as np

os.environ.setdefault("MYCRO_LOCAL_CACHE", "1")
for _p in ("/opt/trn_rl_repo", "/root/.axon_site/_ro/trn_rl_repo"):
    if os.path.isdir(_p) and _p not in sys.path:
        sys.path.append(_p)

B, NQ, NKV = 4, 128, 1024
Q_DIM, KV_DIM, ATTN_DIM = 512, 768, 256
QH = NQ // 2  # 64 queries per core
HH = QH // 2  # 32 queries per post-processing half
N_CORES = 8

_CACHE = {}
last_results = None  # BassKernelResults from the most recent run (for test.py)


def _build():
    from concourse import bacc, masks, mybir, tile

    dt = mybir.dt
    f32 = dt.float32
    bf16 = dt.bfloat16
    AF = mybir.ActivationFunctionType

    nc = bacc.Bacc(
        "TRN2", target_bir_lowering=False, debug=False, num_devices=N_CORES
    )

    q_d = nc.dram_tensor("q", [QH, Q_DIM], f32, kind="ExternalInput").ap()
    kv_d = nc.dram_tensor("kv", [NKV, KV_DIM], f32, kind="ExternalInput").ap()
    wq_d = nc.dram_tensor("wq", [ATTN_DIM, Q_DIM], f32, kind="ExternalInput").ap()
    wk_d = nc.dram_tensor("wk", [ATTN_DIM, KV_DIM], f32, kind="ExternalInput").ap()
    v_d = nc.dram_tensor("v", [1, ATTN_DIM], f32, kind="ExternalInput").ap()
    wv_d = nc.dram_tensor("wv", [Q_DIM, KV_DIM], f32, kind="ExternalInput").ap()
    wo_d = nc.dram_tensor("wo", [Q_DIM, Q_DIM], f32, kind="ExternalInput").ap()
    bo_d = nc.dram_tensor("bo", [1, Q_DIM], f32, kind="ExternalInput").ap()
    alpha_d = nc.dram_tensor("alpha", [QH, NKV], f32, kind="ExternalOutput").ap()
    out_d = nc.dram_tensor("out", [QH, Q_DIM], f32, kind="ExternalOutput").ap()

    CKV = KV_DIM // 128  # 6 c-chunks of kv dim
    CQ = Q_DIM // 128  # 4 c-chunks of q dim
    DB = ATTN_DIM // 128  # 2 d-blocks
    KC = NKV // 128  # 8 k-chunks
    MC = Q_DIM // 128  # 4 m-chunks of context dim

    with tile.TileContext(nc) as tc, ExitStack() as ctx:
        const = ctx.enter_context(tc.tile_pool(name="const", bufs=1))
        persist = ctx.enter_context(tc.tile_pool(name="persist", bufs=1))
        stage = ctx.enter_context(tc.tile_pool(name="stage", bufs=9))
        tpool = ctx.enter_context(tc.tile_pool(name="tanh", bufs=8))
        small = ctx.enter_context(tc.tile_pool(name="small", bufs=1))
        ps_kpt = ctx.enter_context(tc.tile_pool(name="ps_kpt", bufs=1, space="PSUM"))
        ps_et = ctx.enter_context(tc.tile_pool(name="ps_et", bufs=1, space="PSUM"))
        ps_tp = ctx.enter_context(tc.tile_pool(name="ps_tp", bufs=2, space="PSUM"))

        ident = const.tile([128, 128], f32, name="ident", tag="ident")
        masks.make_identity(nc, ident[:])

        # ---- all input DMAs up front, kv (the gating tensor) first ----
        kv_cc = [
            stage.tile([128, NKV], f32, name="kv_cc", tag="kv_cc", bufs=CKV)
            for _ in range(CKV)
        ]
        wk_st = [
            stage.tile([128, KV_DIM], f32, name="stage", tag="stage") for _ in range(DB)
        ]
        for rc in range(DB):
            nc.sync.dma_start(wk_st[rc][:], wk_d[rc * 128 : (rc + 1) * 128, :])
        q_st = stage.tile([QH, Q_DIM], f32, name="stage", tag="stage")
        nc.sync.dma_start(q_st[:], q_d[:])
        wq_st = [
            stage.tile([128, Q_DIM], f32, name="stage", tag="stage") for _ in range(DB)
        ]
        for rc in range(DB):
            nc.sync.dma_start(wq_st[rc][:], wq_d[rc * 128 : (rc + 1) * 128, :])

        # kv arrives column-chunk-major (one big DMA per c-chunk) so the
        # transpose + k_proj chain pipelines with arrival
        for cc in range(CKV):
            nc.sync.dma_start(
                kv_cc[cc][:].rearrange("p (rc c) -> p rc c", c=128),
                kv_d[:, cc * 128 : (cc + 1) * 128].rearrange(
                    "(rc p) c -> p rc c", p=128
                ),
            )
        wv_st = [
            stage.tile([128, KV_DIM], f32, name="stage", tag="stage") for _ in range(4)
        ]
        for rc in range(4):
            nc.sync.dma_start(wv_st[rc][:], wv_d[rc * 128 : (rc + 1) * 128, :])

        v_sb = const.tile([128, DB], f32, name="v_sb", tag="v_sb")
        for db in range(DB):
            nc.sync.dma_start(
                v_sb[:, db : db + 1],
                v_d[0:1, db * 128 : (db + 1) * 128].rearrange("a b -> b a"),
            )
        v_bf = const.tile([128, DB], bf16, name="v_bf", tag="v_bf")
        nc.vector.tensor_copy(v_bf[:], v_sb[:])
        bo_sb = const.tile([1, Q_DIM], f32, name="bo_sb", tag="bo_sb")
        nc.sync.dma_start(bo_sb[:], bo_d[:])
        bo_bf = const.tile([1, Q_DIM], bf16, name="bo_bf", tag="bo_bf")
        nc.vector.tensor_copy(bo_bf[:], bo_sb[:])
        ones_sb = const.tile([1, QH], bf16, name="ones_sb", tag="ones_sb")
        nc.vector.memset(ones_sb[:], 1.0)

        def pe_t(out_ap, in_ap):
            # out = in^T via PE; in_ap [P, M] sbuf -> out_ap [M, P] psum
            kdim = in_ap.shape[0]
            nc.tensor.transpose(out_ap, in_ap, ident[0:kdim, 0:kdim])

        # ---- wk transpose: wkT[cc] [128, 256] bf16 ----
        wkT = [
            persist.tile([128, ATTN_DIM], bf16, name=f"wkT{c}", tag=f"wkT{c}")
            for c in range(CKV)
        ]
        for cc in range(CKV):
            tp = ps_tp.tile([128, 512], f32, name="tp", tag="tp")
            for j in range(DB):
                pe_t(
                    tp[:, j * 128 : (j + 1) * 128],
                    wk_st[j][:, cc * 128 : (cc + 1) * 128],
                )
            nc.scalar.copy(wkT[cc][:], tp[:, 0:ATTN_DIM])

        # ---- q transpose -> qT [128, (4cc x 64)] bf16 ----
        qT = persist.tile([128, CQ * QH], bf16, name="qT", tag="qT")
        tp = ps_tp.tile([128, 512], f32, name="tp", tag="tp")
        for cc in range(CQ):
            pe_t(tp[:, cc * QH : (cc + 1) * QH], q_st[:, cc * 128 : (cc + 1) * 128])
        nc.vector.tensor_copy(qT[:], tp[:, 0 : CQ * QH])

        # ---- wq transpose: wqT[cc] [128, 256] bf16 ----
        wqT = [
            persist.tile([128, ATTN_DIM], bf16, name=f"wqT{c}", tag=f"wqT{c}")
            for c in range(CQ)
        ]
        for cc in range(CQ):
            tp = ps_tp.tile([128, 512], f32, name="tp", tag="tp")
            for j in range(DB):
                pe_t(
                    tp[:, j * 128 : (j + 1) * 128],
                    wq_st[j][:, cc * 128 : (cc + 1) * 128],
                )
            nc.vector.tensor_copy(wqT[cc][:], tp[:, 0:ATTN_DIM])

        # ---- qpT = W_q @ q^T -> [128, (db x 64)] sbuf f32 (ACT bias source) ----
        qpt_ps = ps_tp.tile([128, DB * QH], f32, name="tp", tag="tp")
        for db in range(DB):
            for cc in range(CQ):
                nc.tensor.matmul(
                    qpt_ps[:, db * QH : (db + 1) * QH],
                    lhsT=wqT[cc][:, db * 128 : (db + 1) * 128],
                    rhs=qT[:, cc * QH : (cc + 1) * QH],
                    start=(cc == 0),
                    stop=(cc == CQ - 1),
                )
        qpt = persist.tile([128, DB * QH], f32, name="qpt", tag="qpt")
        nc.vector.tensor_copy(qpt[:], qpt_ps[:])

        # ---- kv transpose + k_proj interleaved per c-chunk ----
        kvT = [
            persist.tile([128, NKV], bf16, name=f"kvT{c}", tag=f"kvT{c}")
            for c in range(CKV)
        ]
        kpt = [
            ps_kpt.tile([128, NKV], f32, name=f"kpt{d}", tag=f"kpt{d}")
            for d in range(DB)
        ]
        for cc in range(CKV):
            for g in range(2):  # groups of 4 row-chunks
                tp = ps_tp.tile([128, 512], f32, name="tp", tag="tp")
                for j in range(4):
                    rc = g * 4 + j
                    pe_t(
                        tp[:, j * 128 : (j + 1) * 128],
                        kv_cc[cc][:, rc * 128 : (rc + 1) * 128],
                    )
                nc.scalar.copy(kvT[cc][:, g * 512 : (g + 1) * 512], tp[:])
            for db in range(DB):
                for nh in range(2):
                    nc.tensor.matmul(
                        kpt[db][:, nh * 512 : (nh + 1) * 512],
                        lhsT=wkT[cc][:, db * 128 : (db + 1) * 128],
                        rhs=kvT[cc][:, nh * 512 : (nh + 1) * 512],
                        start=(cc == 0),
                        stop=(cc == CKV - 1),
                        skip_group_check=True,
                    )

        # ---- wv transpose: wvT[cc] [128, 512] bf16 ----
        wvT = [
            persist.tile([128, Q_DIM], bf16, name=f"wvT{c}", tag=f"wvT{c}")
            for c in range(CKV)
        ]
        for cc in range(CKV):
            tp = ps_tp.tile([128, 512], f32, name="tp", tag="tp")
            for j in range(4):
                pe_t(
                    tp[:, j * 128 : (j + 1) * 128],
                    wv_st[j][:, cc * 128 : (cc + 1) * 128],
                )
            nc.vector.tensor_copy(wvT[cc][:], tp[:])

        # ---- v_proj = kv @ W_v^T -> vproj[kc] [128, 512] bf16 sbuf ----
        # emitted one kc-chunk at a time, interleaved into the h0 tanh loop
        vproj = [
            persist.tile([128, Q_DIM], bf16, name=f"vproj{k}", tag=f"vproj{k}")
            for k in range(KC)
        ]

        def emit_vproj(kc):
            vp = ps_tp.tile([128, 512], f32, name="tp", tag="tp")
            for cc in range(CKV):
                nc.tensor.matmul(
                    vp[:],
                    lhsT=kvT[cc][:, kc * 128 : (kc + 1) * 128],
                    rhs=wvT[cc][:],
                    start=(cc == 0),
                    stop=(cc == CKV - 1),
                )
            nc.vector.tensor_copy(vproj[kc][:], vp[:])

        # ---- main loop in two halves of 32 queries, post overlapped ----
        AFt = AF.Tanh

        def q_half_part(h, qq0, qq1, et=None, filler=None):
            if et is None:
                et = ps_et.tile([128, KC * HH], f32, name=f"et{h}", tag=f"et{h}")
            for qq in range(qq0, qq1):
                qi = h * HH + qq
                tt = []
                for db in range(DB):
                    t = tpool.tile([128, NKV], bf16, name=f"t{db}", tag=f"t{db}")
                    nc.scalar.activation(
                        t[:],
                        kpt[db][:],
                        AFt,
                        bias=qpt[:, db * QH + qi : db * QH + qi + 1],
                    )
                    tt.append(t)
                for kc in range(KC):
                    col = et[:, kc * HH + qq : kc * HH + qq + 1]
                    for db in range(DB):
                        nc.tensor.matmul(
                            col,
                            lhsT=tt[db][:, kc * 128 : (kc + 1) * 128],
                            rhs=v_bf[:, db : db + 1],
                            start=(db == 0),
                            stop=(db == DB - 1),
                        )
                if filler is not None and qq % 4 == 3:
                    filler(qq // 4)
            return et

        def post_half(h, et):
            # transpose energy^T -> energy [32, 1024] (two PSUM halves)
            etsb = persist.tile([128, KC * HH], f32, name=f"etsb{h}", tag=f"etsb{h}")
            nc.vector.tensor_copy(etsb[:], et[:])
            e_ps = []
            for half in range(2):
                ep = ps_tp.tile([HH, 512], f32, name="tp", tag="tp")
                for j in range(4):
                    kc = half * 4 + j
                    pe_t(
                        ep[:, j * 128 : (j + 1) * 128],
                        etsb[:, kc * HH : (kc + 1) * HH],
                    )
                e_ps.append(ep)

            # softmax over k: |energy| <= sum|v_d| ~ 8, so exp cannot
            # overflow in f32 and the max-subtraction is unnecessary
            p_sb = persist.tile([HH, NKV], f32, name=f"p_sb{h}", tag=f"p_sb{h}")
            ssum = small.tile([HH, 2], f32, name=f"ssum{h}", tag=f"ssum{h}")
            for half in range(2):
                nc.scalar.activation(
                    p_sb[:, half * 512 : (half + 1) * 512],
                    e_ps[half][:],
                    AF.Exp,
                    bias=0.0,
                    accum_out=ssum[:, half : half + 1],
                )
            stot = small.tile([HH, 1], f32, name=f"stot{h}", tag=f"stot{h}")
            nc.vector.tensor_reduce(
                stot[:], ssum[:], axis=mybir.AxisListType.X, op=mybir.AluOpType.add
            )
            rinv = small.tile([HH, 1], f32, name=f"rinv{h}", tag=f"rinv{h}")
            nc.vector.reciprocal(rinv[:], stot[:])
            alpha_sb = persist.tile(
                [HH, NKV], f32, name=f"alpha_sb{h}", tag=f"alpha_sb{h}"
            )
            nc.vector.tensor_scalar_mul(alpha_sb[:], p_sb[:], rinv[:])
            nc.sync.dma_start(alpha_d[h * HH : (h + 1) * HH, :], alpha_sb[:])

            # alpha^T -> context = alpha @ v_proj
            a_ps = ps_tp.tile([128, KC * HH], f32, name="tp", tag="tp")
            for kc in range(KC):
                pe_t(
                    a_ps[:, kc * HH : (kc + 1) * HH],
                    alpha_sb[:, kc * 128 : (kc + 1) * 128],
                )
            aT = persist.tile([128, KC * HH], bf16, name=f"aT{h}", tag=f"aT{h}")
            nc.vector.tensor_copy(aT[:], a_ps[:])

            ctx_ps = ps_tp.tile([HH, Q_DIM], f32, name="tp", tag="tp")
            for kc in range(KC):
                nc.tensor.matmul(
                    ctx_ps[:],
                    lhsT=aT[:, kc * HH : (kc + 1) * HH],
                    rhs=vproj[kc][:],
                    start=(kc == 0),
                    stop=(kc == KC - 1),
                )
            ctx_sb = persist.tile([HH, Q_DIM], f32, name=f"ctx_sb{h}", tag=f"ctx_sb{h}")
            nc.vector.tensor_copy(ctx_sb[:], ctx_ps[:])

            # context^T -> out = context @ W_o^T + b_o
            c_ps = ps_tp.tile([128, MC * HH], f32, name="tp", tag="tp")
            for mc in range(MC):
                pe_t(
                    c_ps[:, mc * HH : (mc + 1) * HH],
                    ctx_sb[:, mc * 128 : (mc + 1) * 128],
                )
            cT = persist.tile([128, MC * HH], bf16, name=f"cT{h}", tag=f"cT{h}")
            nc.vector.tensor_copy(cT[:], c_ps[:])

            o_ps = ps_tp.tile([HH, Q_DIM], f32, name="tp", tag="tp")
            for mc in range(MC):
                nc.tensor.matmul(
                    o_ps[:],
                    lhsT=cT[:, mc * HH : (mc + 1) * HH],
                    rhs=woT[mc][:],
                    start=(mc == 0),
                    stop=False,
                )
            nc.tensor.matmul(
                o_ps[:], lhsT=ones_sb[:, 0:HH], rhs=bo_bf[:], start=False, stop=True
            )
            o_sb = persist.tile([HH, Q_DIM], f32, name=f"o_sb{h}", tag=f"o_sb{h}")
            nc.vector.tensor_copy(o_sb[:], o_ps[:])
            nc.sync.dma_start(out_d[h * HH : (h + 1) * HH, :], o_sb[:])

        def wo_chain():
            wo_st = [
                stage.tile([128, Q_DIM], f32, name="wo_stage", tag="wo_stage")
                for _ in range(4)
            ]
            for rc in range(4):
                nc.sync.dma_start(wo_st[rc][:], wo_d[rc * 128 : (rc + 1) * 128, :])
            woT = [
                persist.tile([128, Q_DIM], bf16, name=f"woT{c}", tag=f"woT{c}")
                for c in range(MC)
            ]
            for cc in range(MC):
                tp = ps_tp.tile([128, 512], f32, name="tp", tag="tp")
                for j in range(4):
                    pe_t(
                        tp[:, j * 128 : (j + 1) * 128],
                        wo_st[j][:, cc * 128 : (cc + 1) * 128],
                    )
                nc.vector.tensor_copy(woT[cc][:], tp[:])
            return woT

        et0 = q_half_part(0, 0, HH, filler=emit_vproj)
        woT = wo_chain()
        et1_a = q_half_part(1, 0, 8)
        post_half(0, et0)
        q_half_part(1, 8, HH, et1_a)
        post_half(1, et1_a)

    nc.compile()
    return nc


def _get_nc():
    if "nc" not in _CACHE:
        _CACHE["nc"] = _build()
    return _CACHE["nc"]


def kernel(q, kv, W_q, W_k, v, W_v, W_o, b_o):
    global last_results
    from concourse.bass_utils import run_bass_kernel_spmd

    nc = _get_nc()
    q = np.asarray(q, dtype=np.float32)
    kv = np.asarray(kv, dtype=np.float32)
    common = {
        "wq": np.ascontiguousarray(np.asarray(W_q, dtype=np.float32)),
        "wk": np.ascontiguousarray(np.asarray(W_k, dtype=np.float32)),
        "v": np.ascontiguousarray(np.asarray(v, dtype=np.float32).reshape(1, ATTN_DIM)),
        "wv": np.ascontiguousarray(np.asarray(W_v, dtype=np.float32)),
        "wo": np.ascontiguousarray(np.asarray(W_o, dtype=np.float32)),
        "bo": np.ascontiguousarray(np.asarray(b_o, dtype=np.float32).reshape(1, Q_DIM)),
    }
    in_maps = []
    for core in range(N_CORES):
        b, h = core // 2, core % 2
        in_maps.append(
            {
                "q": np.ascontiguousarray(q[b, h * QH : (h + 1) * QH, :]),
                "kv": np.ascontiguousarray(kv[b]),
                **common,
            }
        )

    trace = bool(os.environ.get("BASS_TRACE"))
    last_results = run_bass_kernel_spmd(
        nc, in_maps, core_ids=list(range(N_CORES)), trace=trace
    )
    res = last_results.results

    out = np.empty((B, NQ, Q_DIM), dtype=np.float32)
    alpha = np.empty((B, NQ, NKV), dtype=np.float32)
    for core in range(N_CORES):
        b, h = core // 2, core % 2
        out[b, h * QH : (h + 1) * QH, :] = res[core]["out"]
        alpha[b, h * QH : (h + 1) * QH, :] = res[core]["alpha"]
    return out, alpha
